# revision 13
# baseline (speedup 1.0000x reference)
"""Multi-head attention (b=2, t=2048, k=1024, 16 heads) on 8 TRN2 NeuronCores.

Sharding: batch across 2 groups of 4 cores; within a group, heads are
tensor-parallel (4 heads/core as 2 head-pairs).  Per-core pipeline (v3):
  1. bf16 projections from pre-transposed x/W.  Only Q/K for head-pair 0 and
     half of head-pair-0's V run up front; the rest trickle into the PE's
     idle slots during the (ACT-bound) attention phase.
  2. attention per (head-pair, 512-query chunk):
       S slot: S.T_A (rows 0-63) + S.T_B (rows 64-127) bf16 -> psum [128,1024]
       exp [128,1024] -> P bf16 (ACT is the critical path: 128 x ~1.07us)
       O slot: per key tile, 2 matmuls [K=128, M=65, N=512]; V carries a
         65th ones-column so each head's psum row 64 is the softmax
         denominator (no separate denominator matmuls)
       normalize: DVE row-64 copy -> reciprocal_approx_fast -> gpsimd
         partition_broadcast -> DVE multiply -> bf16 O.T
  3. per (head-pair, query-half) 4-core AllGather of bf16 O.T (4 gathers)
  4. Wo bf16 matmul (rows permuted on host to gather order) -> y.T slice
"""

import sys

if '/opt/trn_rl_repo' not in sys.path:
    sys.path.insert(0, '/opt/trn_rl_repo')

import ml_dtypes
import numpy as np

B = 2
T = 2048
KD = 1024
NH = 16
HS = 64
NCORES = 8
GROUP = 4                 # cores per batch group
NH_LOC = NH // GROUP      # heads per core
NHP = NH_LOC // 2         # head-pairs per core
TSLICE = T // GROUP       # output tokens per core
HFEAT = NH_LOC * HS       # 256 local head features
NKT = T // 128            # 16 key-token tiles
NKD = KD // 128           # 8 model-feature tiles
NQ4 = T // 512            # 4 query chunks of 512
VD = HS + 1               # V row + ones column (denominator fold)

_CACHE = {}


def _build():
    import concourse.bass as bass
    import concourse.mybir as mybir
    import concourse.tile as tile
    from concourse import bacc

    F32 = mybir.dt.float32
    BF16 = mybir.dt.bfloat16
    AF = mybir.ActivationFunctionType

    nc = bacc.Bacc("TRN2", target_bir_lowering=False, debug=False,
                   num_devices=NCORES)

    xT = nc.dram_tensor("xT", [KD, T], BF16, kind="ExternalInput")
    wqT = nc.dram_tensor("wqT", [KD, HFEAT], BF16, kind="ExternalInput")
    wkT = nc.dram_tensor("wkT", [KD, HFEAT], BF16, kind="ExternalInput")
    wvT = nc.dram_tensor("wvT", [KD, HFEAT], BF16, kind="ExternalInput")
    woT = nc.dram_tensor("woT", [KD, KD], BF16, kind="ExternalInput")
    yT = nc.dram_tensor("yT", [KD, TSLICE], F32, kind="ExternalOutput")

    rgroups = [list(range(GROUP)), list(range(GROUP, 2 * GROUP))]

    with tile.TileContext(nc) as tc:
        with (
            tc.tile_pool(name="big", bufs=1) as big,
            tc.tile_pool(name="pt", bufs=4) as pt_pool,
            tc.tile_pool(name="rbp", bufs=2) as rb_pool,
            tc.tile_pool(name="onp", bufs=4) as on_pool,
            tc.tile_pool(name="ytp", bufs=2) as yt_pool,
            tc.tile_pool(name="dram", bufs=1, space="DRAM") as dram,
        ):
            xt = big.tile([128, NKD, T], BF16)
            wq = big.tile([128, NKD, HFEAT], BF16)
            wk = big.tile([128, NKD, HFEAT], BF16)
            wv = big.tile([128, NKD, HFEAT], BF16)
            wo = big.tile([128, NKD, KD], BF16)
            # rows of qt/kt tile hp: 0-63 = head 2hp, 64-127 = head 2hp+1
            qt = [big.tile([128, T], BF16, name=f"qt{m}") for m in range(2)]
            kt = [big.tile([128, T], BF16, name=f"kt{m}") for m in range(2)]
            # V token-major bf16: [tok%128, kt_tile, head, 65]; col 64 = 1.0
            vp = big.tile([128, NKT, NH_LOC, VD], BF16)
            nc.vector.memset(vp[:, :, :, HS:VD], 1.0)

            # head-pair 0: 2 gathers of [128, 1024] (per query-half)
            agin0 = [dram.tile([128, 2 * TSLICE], BF16, name=f"agin0_{q}")
                     for q in range(2)]
            agout0 = dram.tile([2 * GROUP, 128, 2 * TSLICE], BF16,
                               name="agout0")
            # head-pair 1 is chunk-rotated per core (chunk q4 = (rank+1+j)&3
            # at position j) so each core's own Wo slice never waits on the
            # final collective: 3 gathers (positions 0..2) + a local write of
            # the own chunk into slot 12+rank, where the uniform index
            # formula 4*((rank+3-s)&3)+s lands for s == rank.
            agin1 = [dram.tile([128, TSLICE], BF16, name=f"agin1_{j}")
                     for j in range(3)]
            agout1 = dram.tile([4 * GROUP, 128, TSLICE], BF16,
                               name="agout1")

            # preload the ACT exp table while input DMAs stream
            warm = big.tile([1, 16], F32, name="warm")
            nc.vector.memset(warm[:], 0.0)
            nc.scalar.activation(warm[:], warm[:], AF.Exp, scale=1.0)

            # ---- DMA in (x k-slice first so projections start early) ----
            for k in range(NKD):
                r = slice(128 * k, 128 * (k + 1))
                nc.sync.dma_start(xt[:, k, :], xT.ap()[r, :])
                nc.sync.dma_start(wq[:, k, :], wqT.ap()[r, :])
                nc.sync.dma_start(wk[:, k, :], wkT.ap()[r, :])
                nc.sync.dma_start(wv[:, k, :], wvT.ap()[r, :])

            # ---- phase 1a: kt0 full, qt0 col 0, V head-pair 0 tiles 0..7 ----
            with tc.tile_pool(name="ppsum", bufs=1, space="PSUM") as ppsum:
                acc = [ppsum.tile([128, 512], F32, name=f"acc{i}",
                                  tag=f"acc{i}") for i in range(8)]

                def emit_qk(wtile, m, n, ps, ks):
                    dst = (qt, kt)[0 if wtile is wq else 1]
                    for k in ks:
                        nc.tensor.matmul(
                            ps[:], wtile[:, k, 128 * m:128 * (m + 1)],
                            xt[:, k, 512 * n:512 * (n + 1)],
                            start=(k == 0), stop=(k == NKD - 1),
                        )
                    if ks[-1] == NKD - 1:
                        nc.vector.tensor_copy(
                            dst[m][:, 512 * n:512 * (n + 1)], ps[:])

                # k-outer across 8 psum banks so the PE starts as soon as
                # each xt k-slice lands and never idles (idle resets the
                # p-state ramp): kt0 n0..3 + qt0 n0 + V hp0 tiles 0..2
                for k in range(NKD):
                    for n in range(4):
                        nc.tensor.matmul(
                            acc[n][:], wk[:, k, 0:128],
                            xt[:, k, 512 * n:512 * (n + 1)],
                            start=(k == 0), stop=(k == NKD - 1),
                        )
                    nc.tensor.matmul(
                        acc[4][:], wq[:, k, 0:128],
                        xt[:, k, 0:512],
                        start=(k == 0), stop=(k == NKD - 1),
                    )
                    for mtv in range(3):
                        nc.tensor.matmul(
                            acc[5 + mtv][:, 0:128],
                            xt[:, k, 128 * mtv:128 * (mtv + 1)],
                            wv[:, k, 0:128],
                            start=(k == 0), stop=(k == NKD - 1),
                        )
                for n in range(4):
                    nc.vector.tensor_copy(
                        kt[0][:, 512 * n:512 * (n + 1)], acc[n][:])
                nc.vector.tensor_copy(qt[0][:, 0:512], acc[4][:])
                for mtv in range(3):
                    nc.vector.tensor_copy(
                        vp[:, mtv, 0:2, 0:HS],
                        acc[5 + mtv][:, 0:128].rearrange(
                            "p (h d) -> p h d", h=2),
                    )

                def emit_v(mt, hp, ps, ks=None):
                    # V for one head-pair, one 128-token tile
                    fs = slice(128 * hp, 128 * (hp + 1))
                    ks = range(NKD) if ks is None else ks
                    ret = None
                    for k in ks:
                        nc.tensor.matmul(
                            ps[:, 0:128],
                            xt[:, k, 128 * mt:128 * (mt + 1)],
                            wv[:, k, fs],
                            start=(k == 0), stop=(k == NKD - 1),
                        )
                    if list(ks)[-1] == NKD - 1:
                        ret = nc.vector.tensor_copy(
                            vp[:, mt, 2 * hp:2 * hp + 2, 0:HS],
                            ps[:, 0:128].rearrange("p (h d) -> p h d", h=2),
                        )
                    return ret

                for mt in range(3, 8):
                    vcopy = emit_v(mt, 0, ppsum.tile([128, 512], F32,
                                                     name=f"vps{mt}",
                                                     tag=f"acc{mt % 8}"))

            # wo prefetch deferred until the projections' DMAs are queued
            for k in range(NKD):
                wdma = nc.sync.dma_start(
                    wo[:, k, :], woT.ap()[128 * k:128 * (k + 1), :])
                tile.add_dep_helper(vcopy.ins, wdma.ins, sync=False,
                                    reason="defer wo prefetch")

            # ---- phase 2: attention ----
            pid = nc.partition_id()
            qoff1 = [(pid + 1 + j) & 3 for j in range(NQ4)]
            with (
                tc.tile_pool(name="spsum", bufs=2, space="PSUM") as spsum,
                tc.tile_pool(name="odsum", bufs=1, space="PSUM") as odsum,
                tc.tile_pool(name="ospool", bufs=2) as os_pool,
                tc.tile_pool(name="aux", bufs=2, space="PSUM") as aux,
            ):
                v_ps = {}

                def t_v(mt, hp, half=None):
                    def f():
                        if half is None:
                            emit_v(mt, hp, aux.tile([128, 512], F32,
                                                    name=f"vtk{hp}_{mt}",
                                                    tag="aux"), range(NKD))
                            return
                        if half == 0:
                            v_ps[(mt, hp)] = aux.tile([128, 512], F32,
                                                      name=f"vtk{hp}_{mt}",
                                                      tag="aux")
                        emit_v(mt, hp, v_ps[(mt, hp)],
                               range(4 * half, 4 * half + 4))
                    return f

                qk_ps = {}

                def t_qk(wtile, m, n, quarter):
                    def f():
                        key = (id(wtile), m, n)
                        if quarter == 0:
                            qk_ps[key] = aux.tile([128, 512], F32,
                                                  name=f"qkt{m}{n}",
                                                  tag="aux")
                        ks = range(2 * quarter, 2 * quarter + 2)
                        emit_qk(wtile, m, n, qk_ps[key], list(ks))
                    return f

                # deadline-scheduled projection trickle: slot -> closures
                sched = {}

                def at(slot, fn):
                    sched.setdefault(slot, []).append(fn)

                for j in range(8):                    # V hp0 tiles 8..15
                    at(2 * j, t_v(8 + j, 0, 0))       # halves, spread thin
                    at(2 * j + 1, t_v(8 + j, 0, 1))
                # quarters (2 matmuls each); head-pair-1 Q/K must be emitted
                # before slot 62, qt1 cols 2/3 slide into hp1's quiet half
                # rotation note: hp1 S reads a per-core qt1 column at
                # every position, so ALL qt1/kt1 columns must be emitted
                # before the first hp1 S-prefetch (slot 62)
                # deadlines: qt0 col n by slot 16n-2 (chunk-n S prefetch);
                # all qt1/kt1 columns by slot 62 (rotation: any column at
                # position 0)
                qsched = [
                    ((wq, 0, 1), (9, 10, 11, 12)),
                    ((wq, 0, 2), (16, 17, 18, 19)),
                    ((wq, 0, 3), (21, 23, 25, 27)),
                    ((wq, 1, 0), (28, 29, 30, 31)),
                    ((wk, 1, 0), (32, 33, 34, 35)),
                    ((wk, 1, 1), (36, 37, 38, 39)),
                    ((wk, 1, 2), (40, 41, 42, 43)),
                    ((wk, 1, 3), (44, 45, 46, 47)),
                    ((wq, 1, 1), (48, 49, 50, 51)),
                    ((wq, 1, 2), (52, 53, 54, 55)),
                    ((wq, 1, 3), (56, 57, 58, 59)),
                ]
                for (w_, m_, n_), sl in qsched:
                    for q_, s_ in enumerate(sl):
                        at(s_, t_qk(w_, m_, n_, q_))
                for j in range(NKT):                  # V hp1, just-in-time
                    at(64 + j, t_v(j, 1))

                # flat slot list: (hp, q4, ktile)
                slots = [(hp, q4, kk) for hp in range(NHP)
                         for q4 in range(NQ4) for kk in range(NKT)]

                ops = {}     # chunk -> (opA, opB)
                pts = {}     # slot index -> pt tile
                sps = {}     # slot index -> sp psum tile

                def emit_s(i):
                    hp, q4, kk = slots[i]
                    if hp == 0:
                        qs = slice(512 * q4, 512 * (q4 + 1))
                    else:
                        qs = bass.ds(qoff1[q4] * 512, 512)
                    ks = slice(128 * kk, 128 * (kk + 1))
                    sp = spsum.tile([128, 1024], F32, name=f"sp{i}",
                                    tag="sp")
                    nc.tensor.matmul(
                        sp[:, 0:512], kt[hp][0:64, ks],
                        qt[hp][0:64, qs], start=True, stop=True,
                        tile_position=(0, 0))
                    nc.tensor.matmul(
                        sp[:, 512:1024], kt[hp][64:128, ks],
                        qt[hp][64:128, qs], start=True, stop=True,
                        tile_position=(64, 0))
                    sps[i] = sp

                def emit_od(i):
                    hp, q4, kk = slots[i]
                    c = 4 * hp + q4
                    if kk == 0:
                        ops[c] = (odsum.tile([VD, 512], F32, name=f"opA{c}",
                                             tag="opA"),
                                  odsum.tile([VD, 512], F32, name=f"opB{c}",
                                             tag="opB"))
                    opA, opB = ops[c]
                    hA, hB = 2 * hp, 2 * hp + 1
                    nc.tensor.matmul(
                        opA[:], vp[:, kk, hA, :], pts[i][:, 0:512],
                        start=(kk == 0), stop=(kk == NKT - 1))
                    nc.tensor.matmul(
                        opB[:], vp[:, kk, hB, :], pts[i][:, 512:1024],
                        start=(kk == 0), stop=(kk == NKT - 1))
                    if kk == NKT - 1:
                        finalize(hp, q4, opA, opB)

                def finalize(hp, q4, opA, opB):
                    last = (hp == NHP - 1 and q4 == NQ4 - 1)
                    d1 = rb_pool.tile([1, 1024], F32, tag="d1")
                    if last:
                        # nothing reuses the op psum banks: skip staging and
                        # read psum directly (shortest critical chain)
                        nc.vector.tensor_copy(d1[:, 0:512], opA[HS:VD, :])
                        nc.vector.tensor_copy(d1[:, 512:1024], opB[HS:VD, :])
                        osA, osB = opA[0:HS, :], opB[0:HS, :]
                    else:
                        # stage psum -> SBUF fast so the next chunk's od can
                        # reuse the op psum banks without waiting on normalize
                        os = os_pool.tile([VD, 1024], F32, tag="os")
                        nc.vector.tensor_copy(os[:, 0:512], opA[:])
                        nc.vector.tensor_copy(os[:, 512:1024], opB[:])
                        nc.vector.tensor_copy(d1[:], os[HS:VD, :])
                        osA, osB = os[0:HS, 0:512], os[0:HS, 512:1024]
                    rb1 = rb_pool.tile([1, 1024], F32, tag="rb1")
                    nc.vector.reciprocal_approx_fast(rb1[:], d1[:])
                    rb64 = rb_pool.tile([64, 1024], F32, tag="rb64")
                    nc.gpsimd.partition_broadcast(rb64[:], rb1[:])
                    onorm = on_pool.tile([128, 512], BF16, tag="on")
                    nc.vector.tensor_mul(onorm[0:64, :], osA, rb64[:, 0:512])
                    nc.vector.tensor_mul(onorm[64:128, :], osB,
                                         rb64[:, 512:1024])
                    if hp == 0:
                        qh, qo = divmod(q4, 2)
                        nc.sync.dma_start(
                            agin0[qh][:, 512 * qo:512 * (qo + 1)], onorm[:])
                        if qo == 1:
                            nc.gpsimd.collective_compute(
                                "AllGather",
                                mybir.AluOpType.bypass,
                                replica_groups=rgroups,
                                ins=[agin0[qh].opt()],
                                outs=[agout0[GROUP * qh:GROUP * (qh + 1),
                                             :, :].opt()],
                            )
                    elif q4 < 3:      # q4 is the position index for hp1
                        nc.sync.dma_start(agin1[q4][:], onorm[:])
                        nc.gpsimd.collective_compute(
                            "AllGather",
                            mybir.AluOpType.bypass,
                            replica_groups=rgroups,
                            ins=[agin1[q4].opt()],
                            outs=[agout1[GROUP * q4:GROUP * (q4 + 1),
                                         :, :].opt()],
                        )
                    else:             # own chunk: local write, no collective
                        nc.sync.dma_start(
                            agout1[bass.ds(12 + (pid & 3), 1),
                                   :, :].squeeze(0),
                            onorm[:])

                emit_s(0)
                emit_s(1)
                for i in range(len(slots)):
                    pt = pt_pool.tile([128, 1024], BF16, name=f"pt{i}",
                                      tag="pt")
                    pts[i] = pt
                    nc.scalar.activation(pt[:], sps.pop(i)[:], AF.Exp,
                                         scale=0.03125)
                    if i >= 1:
                        emit_od(i - 1)
                    if i + 2 < len(slots):
                        emit_s(i + 2)
                    for fn in sched.pop(i, ()):
                        fn()
                emit_od(len(slots) - 1)

            # ---- phase 3: Wo ----
            with tc.tile_pool(name="ypsum", bufs=1, space="PSUM") as ypsum:
                rank2 = pid & 2          # = 2*qhalf of this core's slice
                colo = (pid & 1) * 512   # token-column offset in the q-half
                rhs = big.tile([128, NKD, TSLICE], BF16)
                # hp1 rhs slots ordered by readiness (wo columns are permuted
                # per-core on the host to match): gather j0, j1, own, j2
                for k in range(4):
                    nc.sync.dma_start(
                        rhs[:, k, :],
                        agout0[bass.ds(2 * rank2 + k, 1), :,
                               bass.ds(colo, TSLICE)].squeeze(0),
                    )
                hp1_idx = [0 + ((pid + 3) & 3), 4 + ((pid + 2) & 3),
                           12 + (pid & 3), 8 + ((pid + 1) & 3)]
                for p in range(4):
                    nc.sync.dma_start(
                        rhs[:, 4 + p, :],
                        agout1[bass.ds(hp1_idx[p], 1), :, :].squeeze(0),
                    )
                yps = [ypsum.tile([128, TSLICE], F32, name=f"yp{m}",
                                  tag=f"yp{m}") for m in range(8)]
                # k 0..6 are ready before attention ends (hp0 gathers, hp1
                # gathers j0/j1, own chunk); only k=7 waits on the last
                # gather, so it goes m-outer with the stores pipelined in
                for k in range(NKD - 1):
                    for m in range(8):
                        nc.tensor.matmul(
                            yps[m][:], wo[:, k, 128 * m:128 * (m + 1)],
                            rhs[:, k, :], start=(k == 0), stop=False,
                        )
                for m in range(8):
                    nc.tensor.matmul(
                        yps[m][:], wo[:, NKD - 1, 128 * m:128 * (m + 1)],
                        rhs[:, NKD - 1, :], start=False, stop=True,
                    )
                    yt_s = yt_pool.tile([128, TSLICE], F32, tag="yt")
                    nc.vector.tensor_copy(yt_s[:], yps[m][:])
                    nc.sync.dma_start(yT.ap()[128 * m:128 * (m + 1), :],
                                      yt_s[:])

    nc.compile()
    return nc


def _get_nc():
    if "nc" not in _CACHE:
        _CACHE["nc"] = _build()
    return _CACHE["nc"]


def _make_in_maps(x, Wq, Wk, Wv, Wo):
    # Wo rows permuted to the per-core rhs assembly order.
    # hp0 half (rows 0..511): feature tile k = s (source rank), rows =
    #   (head-in-pair a, dim d) -> true head 4s+0+a.
    # hp1 half (rows 512..1023): tiles ordered by gather readiness for this
    #   core: sources [(rank-1)&3, (rank-2)&3, rank, (rank+1)&3]; head
    #   4s+2+a.
    bf = ml_dtypes.bfloat16
    r = np.arange(128)
    a, d = r // HS, r % HS

    in_maps = []
    for c in range(NCORES):
        g, rr = c // GROUP, c % GROUP
        rows = slice(rr * HFEAT, (rr + 1) * HFEAT)
        perm = np.empty(KD, dtype=np.int64)
        for k in range(4):
            perm[128 * k:128 * (k + 1)] = (GROUP * k + 0 + a) * HS + d
        hp1_src = [(rr - 1) % 4, (rr - 2) % 4, rr, (rr + 1) % 4]
        for p, s in enumerate(hp1_src):
            perm[512 + 128 * p:512 + 128 * (p + 1)] = \
                (GROUP * s + 2 + a) * HS + d
        woTp = np.ascontiguousarray(Wo.T[perm]).astype(bf)
        in_maps.append({
            "xT": np.ascontiguousarray(x[g].T).astype(bf),
            "wqT": np.ascontiguousarray(Wq[rows].T).astype(bf),
            "wkT": np.ascontiguousarray(Wk[rows].T).astype(bf),
            "wvT": np.ascontiguousarray(Wv[rows].T).astype(bf),
            "woT": woTp,
        })
    return in_maps


def kernel(x, Wq, Wk, Wv, Wo):
    from concourse import bass_utils

    x = np.asarray(x, dtype=np.float32)
    Wq = np.asarray(Wq, dtype=np.float32)
    Wk = np.asarray(Wk, dtype=np.float32)
    Wv = np.asarray(Wv, dtype=np.float32)
    Wo = np.asarray(Wo, dtype=np.float32)

    nc = _get_nc()
    in_maps = _make_in_maps(x, Wq, Wk, Wv, Wo)
    res = bass_utils.run_bass_kernel_spmd(nc, in_maps,
                                          core_ids=list(range(NCORES)))

    out = np.empty((B, T, KD), dtype=np.float32)
    for c in range(NCORES):
        g, r = c // GROUP, c % GROUP
        out[g, r * TSLICE:(r + 1) * TSLICE, :] = res.results[c]["yT"].T
    return out
